# revision 44
# baseline (speedup 1.0000x reference)
"""Causal MHA + RoPE (B=2, T=2048, D=2048, H=16, HD=128), fp32 in/out.

Tensor-parallel over heads across 8 NeuronCores (2 heads/core):
  - w_q/w_k/w_v column-sharded (rows of W), w_o row-sharded; partial
    outputs summed on the host.
  - All matmul operands are bf16 (fp32 PSUM accumulation), except the
    off-diagonal attention*V and denominator matmuls which run fp8e4
    DoubleRow (two key tiles per instruction, 2x PE throughput).
    Measured max-rel error vs the fp32 reference ~4e-3 (gate 2e-2).
  - Transposed activation layout ([feature, token]) throughout:
      qT/kT   = W_slice @ x^T              ([HD, T] per head)
      S^T     = kT.T-slice @ qT            ([tk, tq], contraction over HD)
      E       = exp(S^T*scale - 2)         (offset cancels in softmax,
                                            keeps fp8 E under e4m3 max)
      denom   = ones.T @ E                 (cross-partition sum on PE)
      O^T    += (16 v_tile).T @ E          (v scaled x16 out of fp8
                                            subnormals; divided out via
                                            the reciprocal's bias)
      partial = w_oT_slice.T @ OcatT       ([D, T] per batch, per core)
  - V is produced token-major directly (lhsT = x token-block, rhs = w_v
    slice): no PE transposes.
  - x/weights live in SBUF split into separate tiles per DMA chunk
    (Tile dependencies are whole-tile, so separate tiles let the first
    matmuls start after ~0.25 MB instead of the full buffer).
  - RoPE: q/k weight rows pre-permuted on the host (even idx -> top 64
    partitions, odd -> bottom), so the rotation is a half-swap plus
    elementwise mul/add against fp32 cos/sin tables.
"""

import numpy as np

B, T, D, H = 2, 2048, 2048, 16
HD = D // H  # 128
NCORES = 8
HPC = H // NCORES  # heads per core = 2
CD = HPC * HD  # per-core head dims = 256
SCALE = 1.0 / float(np.sqrt(HD))
TB = 512  # token block (matmul free dim)
NTB = T // TB  # 4 token blocks per batch
NKT = T // 128  # 16 key tiles per batch
KO = D // 128  # 16 contraction tiles over D
EXP_OFF = -2.0  # exp offset, cancels in softmax normalization
LN16 = float(np.log(16.0))


_PATCHED = False


def _apply_tile_patches():
    """This container's walrus build allows only ONE sync-wait command per
    TPB instruction (e.g. the S3_LW struct of a fused fp32 matmul rejects
    2 waits with "Too many sync wait commands"). Tile's scheduler freely
    puts several waits on one instruction. Two patches:

    1. After wait assignment, hoist all-but-one waits of every instruction
       onto injected same-engine NoOps placed just before it.
    2. The final TileContext drain aggregates all outstanding waits onto
       one SP Drain — split into a chain of single-wait drains.
    """
    global _PATCHED
    if _PATCHED:
        return
    _PATCHED = True

    import concourse.mybir as mybir
    import concourse.tile as tile
    from concourse.vector_clock import ScopedClock

    MAXW = 1

    _orig_lower = tile.TileContext._lower_ordered_insts

    def _lower_ordered_insts(self, ordered):
        nc = self.nc
        for insts in ordered.values():
            need = any(
                i.sync_info is not None and len(i.sync_info.on_wait) > MAXW
                for i in insts
            )
            if not need:
                continue
            out = []
            for inst in insts:
                si = inst.sync_info
                if si is not None and len(si.on_wait) > MAXW:
                    waits = list(si.on_wait)
                    extra = waits[MAXW:]
                    del si.on_wait[MAXW:]
                    for j in range(0, len(extra), MAXW):
                        nop = mybir.InstNoOp(
                            name=nc.get_next_instruction_name(), ins=[], outs=[]
                        )
                        nop.engine = inst.engine
                        nop.sync_info = mybir.SyncInfo(
                            on_wait=extra[j : j + MAXW], on_update=[]
                        )
                        nc.register_instruction(nop)
                        out.append(nop)
                out.append(inst)
            insts[:] = out
        return _orig_lower(self, ordered)

    def _drain_and_barrier(self, tick_clock, wait_clock):
        drain_inst = self.nc.sync.drain()
        wait_clock.add_sem_waits(
            drain_inst.ins, ScopedClock({None: tick_clock.global_clock})
        )
        si = drain_inst.ins.sync_info
        waits = list(si.on_wait) if si is not None else []
        if len(waits) > 1:
            del si.on_wait[1:]
            for w in waits[1:]:
                extra = self.nc.sync.drain()
                extra.ins.sync_info = mybir.SyncInfo(on_wait=[w], on_update=[])
        self.nc.all_engine_barrier()
        assert self.sems is not None
        popped = self.nc._tile_sem_poison_stack.pop()
        assert popped is self._sem_poison
        self.nc.clear_and_free_semaphores(list(self.sems.allocated().values()))
        self.nc.all_engine_barrier()

    tile.TileContext._lower_ordered_insts = _lower_ordered_insts
    tile.TileContext._drain_and_barrier = _drain_and_barrier


def build_bass():
    _apply_tile_patches()
    import concourse.bass as bass
    import concourse.mybir as mybir
    import concourse.tile as tile

    f32 = mybir.dt.float32
    bf16 = mybir.dt.bfloat16
    f8 = mybir.dt.float8e4
    EXP = mybir.ActivationFunctionType.Exp
    COPY = mybir.ActivationFunctionType.Copy
    DR = mybir.MatmulPerfMode.DoubleRow

    nc = bass.Bass("TRN2", target_bir_lowering=False, debug=False)

    xT = nc.dram_tensor("xT", [B, D, T], bf16, kind="ExternalInput").ap()
    wqT = nc.dram_tensor("wqT", [D, CD], bf16, kind="ExternalInput").ap()
    wkT = nc.dram_tensor("wkT", [D, CD], bf16, kind="ExternalInput").ap()
    wvT = nc.dram_tensor("wvT", [D, CD], bf16, kind="ExternalInput").ap()
    woT = nc.dram_tensor("woT", [CD, D], bf16, kind="ExternalInput").ap()
    cosd = nc.dram_tensor("cosd", [HD, T], bf16, kind="ExternalInput").ap()
    sind = nc.dram_tensor("sind", [HD, T], bf16, kind="ExternalInput").ap()
    out = nc.dram_tensor("out", [B, D, T], bf16, kind="ExternalOutput").ap()

    # weight/x chunk boundaries: separate tiles per chunk (Tile deps are
    # whole-tile; readers of an early chunk must not wait on later DMAs)
    WCH = ((0, 2), (2, 8), (8, KO))
    XCH0 = ((0, 4), (4, 10), (10, KO))  # first token block of a batch, finer

    with tile.TileContext(nc) as tc:
        with (
            tc.tile_pool(name="consts", bufs=1) as cpool,
            tc.tile_pool(name="acts", bufs=1) as apool,
            tc.tile_pool(name="rt", bufs=4) as rpool,
            tc.tile_pool(name="et", bufs=6) as epool,
            tc.tile_pool(name="e8t", bufs=4) as e8pool,
            tc.tile_pool(name="rc", bufs=2) as rcpool,
            tc.tile_pool(name="oc", bufs=2) as ocpool,
            tc.tile_pool(name="obp", bufs=3) as obpool,
            tc.tile_pool(name="ps", bufs=8, space="PSUM") as psp,
        ):
            # ---- persistent constants ----
            wq_t = [cpool.tile([128, k1 - k0, CD], bf16, name=f"wq{k0}") for k0, k1 in WCH]
            wk_t = [cpool.tile([128, k1 - k0, CD], bf16, name=f"wk{k0}") for k0, k1 in WCH]
            wv_sb = cpool.tile([128, KO, CD], bf16, name="wv_sb")
            wo_sb = cpool.tile([128, HPC, D], bf16, name="wo_sb")
            cos_sb = cpool.tile([128, T], bf16, name="cos_sb")
            sin_sb = cpool.tile([128, T], bf16, name="sin_sb")
            ones_sb = cpool.tile([128, 128], bf16, name="ones_sb")
            nc.vector.memset(ones_sb[:], 1.0)
            ones8 = cpool.tile([128, 2, 128], f8, name="ones8")
            nc.vector.memset(ones8[:].bitcast(mybir.dt.uint8), 0x38)  # fp8e4 1.0
            # const AP for the exp offset bias
            cap = cpool.tile([128, 1], f32, name="cexpoff")
            nc.vector.memset(cap[:], EXP_OFF)
            nc.const_aps.aps[(f32, EXP_OFF)] = cap[:]
            # warm-up matmuls: no DMA deps, ramp the PE clock to full
            # speed while the DMA engines spin up
            warm_sb = cpool.tile([128, TB], bf16, name="warm_sb")
            nc.vector.memset(warm_sb[:], 0.0)
            warm_ps = psp.tile([128, TB], f32, name="warm_ps", tag="ps")
            for _ in range(24):
                nc.tensor.matmul(
                    warm_ps[:],
                    lhsT=ones_sb[:],
                    rhs=warm_sb[:],
                    start=True,
                    stop=True,
                    skip_group_check=True,
                )

            def wqk_at(ko):
                for ci, (k0, k1) in enumerate(WCH):
                    if k0 <= ko < k1:
                        return wq_t[ci], wk_t[ci], ko - k0
                raise AssertionError

            # q/k weights stream chunk-interleaved on the ACT queue so
            # slice ko stays ahead of the matmul pace; v/o weights and
            # rope tables ride the SWDGE queue (needed later). The DMA
            # engines round-robin descriptors across every enqueued
            # transfer, so anything enqueued early steals bandwidth from
            # the first chunks — keep each queue's early contents minimal
            # and in need-order.
            def load_wqk_chunk(ci, eng):
                k0, k1 = WCH[ci]
                for w_t, src in ((wq_t[ci], wqT), (wk_t[ci], wkT)):
                    eng.dma_start(
                        w_t[:],
                        src[k0 * 128 : k1 * 128, :].rearrange(
                            "(ko p) n -> p ko n", p=128
                        ),
                    )

            def load_wv_chunk(ci):
                k0, k1 = WCH[ci]
                nc.gpsimd.dma_start(
                    wv_sb[:, k0:k1, :],
                    wvT[k0 * 128 : k1 * 128, :].rearrange(
                        "(ko p) n -> p ko n", p=128
                    ),
                )

            load_wqk_chunk(0, nc.scalar)
            load_wqk_chunk(1, nc.scalar)
            load_wv_chunk(0)
            load_wqk_chunk(2, nc.gpsimd)
            load_wv_chunk(1)
            load_wv_chunk(2)
            nc.gpsimd.dma_start(cos_sb[:], cosd)
            nc.gpsimd.dma_start(sin_sb[:], sind)
            nc.gpsimd.dma_start(
                wo_sb[:], woT.rearrange("(kk p) n -> p kk n", p=128)
            )

            # ---- per-batch activation storage (slots reused across batches) ----
            # x: one tile per DMA chunk; first token block split finer
            xb_t = {}
            for nb in range(NTB):
                chunks = XCH0 if nb == 0 else ((0, KO),)
                for h0, h1 in chunks:
                    xb_t[nb, h0, h1] = apool.tile(
                        [128, h1 - h0, TB], bf16, name=f"xb{nb}_{h0}"
                    )

            def xb_at(nb, ko):
                chunks = XCH0 if nb == 0 else ((0, KO),)
                for h0, h1 in chunks:
                    if h0 <= ko < h1:
                        return xb_t[nb, h0, h1], ko - h0
                raise AssertionError

            qT_sb = apool.tile([128, HPC, T], bf16, name="qT_sb")
            kT_sb = apool.tile([128, HPC, T], bf16, name="kT_sb")
            vh_sb = apool.tile([128, NKT, CD], bf16, name="vh_sb")
            vh8 = apool.tile([128, NKT, CD], f8, name="vh8")

            def ps_tile(nm, w=TB):
                return psp.tile([128, w], f32, name=nm, tag="ps")

            def load_x(b):
                # sync carries only what's needed soonest (nb0 low/mid,
                # then nb2/nb3); nb0-high and all of nb1 ride the ACT
                # queue interleaved with the q/k weight chunks, in
                # need-order, so no transfer is starved by round-robin
                for h0, h1 in ((0, 4), (4, 10)):
                    nc.sync.dma_start(
                        xb_t[0, h0, h1][:],
                        xT[b, h0 * 128 : h1 * 128, 0:TB].rearrange(
                            "(ko p) t -> p ko t", p=128
                        ),
                    )
                nc.scalar.dma_start(
                    xb_t[0, 10, KO][:],
                    xT[b, 10 * 128 : KO * 128, 0:TB].rearrange(
                        "(ko p) t -> p ko t", p=128
                    ),
                )
                nc.scalar.dma_start(
                    xb_t[1, 0, KO][:],
                    xT[b, :, TB : 2 * TB].rearrange("(ko p) t -> p ko t", p=128),
                )
                for nb in (2, 3):
                    nc.sync.dma_start(
                        xb_t[nb, 0, KO][:],
                        xT[b, :, nb * TB : (nb + 1) * TB].rearrange(
                            "(ko p) t -> p ko t", p=128
                        ),
                    )

            # pending projection work: list of thunks, each emits one
            # dout (two matmuls + copy, one store per `stg` douts)
            pending = []
            drain_on_act = [False]  # psum->sbuf copy engine for drains

            def emit_proj_block(bb, jj, ocb, stg=4, alt=False):
                tqp = slice(jj * TB, (jj + 1) * TB)
                obg = {}

                def mk(do):
                    def thunk():
                        pp = ps_tile("pp")
                        for kk in range(HPC):
                            nc.tensor.matmul(
                                pp[:],
                                lhsT=wo_sb[:, kk, do * 128 : (do + 1) * 128],
                                rhs=ocb[:, kk, :],
                                start=(kk == 0),
                                stop=(kk == HPC - 1),
                                skip_group_check=True,
                            )
                        if do % stg == 0:
                            obg["t"] = obpool.tile(
                                [128, stg, TB], bf16, name="ob", tag="ob"
                            )
                        ob = obg["t"]
                        if drain_on_act[0] or (alt and do % 2):
                            nc.scalar.copy(ob[:, do % stg, :], pp[:])
                        else:
                            nc.vector.tensor_copy(ob[:, do % stg, :], pp[:])
                        if do % stg == stg - 1:
                            seng = nc.gpsimd if bb == 0 else nc.sync
                            seng.dma_start(
                                out[
                                    bb,
                                    (do - stg + 1) * 128 : (do + 1) * 128,
                                    tqp,
                                ].rearrange("(g p) t -> p g t", p=128),
                                ob[:],
                            )

                    return thunk

                for do in range(D // 128):
                    pending.append(mk(do))

            def drain_pending(k):
                for _ in range(min(k, len(pending))):
                    pending.pop(0)()

            for b in range(B):
                load_x(b)
                # ============ QKV projections (+RoPE), V token-major ============
                for nb in range(NTB):
                    tsl = slice(nb * TB, (nb + 1) * TB)
                    psums = {}
                    for w in ("q", "k"):
                        for m in range(HPC):
                            psums[w, m] = ps_tile(f"ps_{w}{m}")
                    drain_on_act[0] = True
                    for ko in range(KO):
                        wq_c, wk_c, lko = wqk_at(ko)
                        xt, xko = xb_at(nb, ko)
                        for w, w_c in (("q", wq_c), ("k", wk_c)):
                            for m in range(HPC):
                                nc.tensor.matmul(
                                    psums[w, m][:],
                                    lhsT=w_c[:, lko, m * 128 : (m + 1) * 128],
                                    rhs=xt[:, xko, :],
                                    start=(ko == 0),
                                    stop=(ko == KO - 1),
                                )
                        if nb == 0 and ko in (5, 9, 13):
                            drain_pending(6)
                    drain_on_act[0] = False
                    # RoPE for q, k -> SBUF (all-DVE, partition-shifted
                    # reads; fp32 temps, single bf16 rounding at the add).
                    # Complete chains per head so qT/kT finish on DVE
                    # before the vh8 copies queue behind them — the
                    # attention block right below waits on these.
                    for w, dst in (("q", qT_sb), ("k", kT_sb)):
                        for m in range(HPC):
                            ps = psums[w, m]
                            tmp = rpool.tile([128, TB], f32, name="rtmp", tag="rtmp")
                            d = dst[:, m, tsl]
                            nc.vector.tensor_mul(d, ps[:], cos_sb[:, tsl])
                            nc.vector.tensor_mul(
                                tmp[0:64, :], ps[64:128, :], sin_sb[0:64, tsl]
                            )
                            nc.vector.tensor_mul(
                                tmp[64:128, :], ps[0:64, :], sin_sb[64:128, tsl]
                            )
                            nc.vector.tensor_add(d, d, tmp[:])
                    # V token-major: lhsT = x token block, rhs = w_v slice.
                    # Both copies scale v by 16 (fp8 subnormal headroom),
                    # divided back out via w_o/16 on the host.
                    for tkb in range(4):
                        pv = ps_tile("pv", w=CD)
                        for ko in range(KO):
                            xt, xko = xb_at(nb, ko)
                            nc.tensor.matmul(
                                pv[:],
                                lhsT=xt[:, xko, tkb * 128 : (tkb + 1) * 128],
                                rhs=wv_sb[:, ko, :],
                                start=(ko == 0),
                                stop=(ko == KO - 1),
                            )
                        kt = nb * 4 + tkb
                        nc.scalar.activation(
                            vh_sb[:, kt, :], pv[:], COPY, scale=16.0
                        )
                        # gpsimd (idle, SBUF-only) so the fp8 copy isn't
                        # queued behind the rope chain on DVE
                        nc.gpsimd.tensor_copy(vh8[:, kt, :], vh_sb[:, kt, :])

                    # ======== attention block j4 = nb (keys 0..nb ready) ========
                    # interleaved with QKV so the PE stays dense while x
                    # and weights stream in, and so block-end divide
                    # chains overlap the next block's projections
                    j4 = nb
                    tq = slice(j4 * TB, (j4 + 1) * TB)
                    n_tk = 4 * (j4 + 1)
                    n_off = 4 * j4  # off-diagonal tiles (fp8 DoubleRow pairs)
                    ocb = ocpool.tile([128, HPC, TB], bf16, name="ocb", tag="ocb")
                    o_ps = [ps_tile(f"o_ps{h}") for h in range(HPC)]
                    den_ps = [ps_tile(f"den_ps{h}") for h in range(HPC)]
                    e8cur = {}

                    def s_mm(h, i):
                        s = ps_tile("s_ps")
                        p = i - n_off
                        c0 = 128 * p if p > 0 else 0
                        nc.tensor.matmul(
                            s[:, c0:],
                            lhsT=kT_sb[:, h, i * 128 : (i + 1) * 128],
                            rhs=qT_sb[:, h, j4 * TB + c0 : (j4 + 1) * TB],
                            start=True,
                            stop=True,
                            skip_group_check=True,
                        )
                        return s

                    def exp_tile(h, i, s):
                        p = i - n_off
                        if p < 0:
                            # off-diagonal: fp8 pair slot for DoubleRow
                            if i % 2 == 0:
                                e8cur[h] = e8pool.tile(
                                    [128, 2, TB], f8, name="e8", tag="e8"
                                )
                            e_sb = e8cur[h][:, i % 2, :]
                            nc.scalar.activation(
                                e_sb, s[:], EXP, scale=SCALE, bias=EXP_OFF
                            )
                            return e_sb
                        e_sb = epool.tile([128, TB], bf16, name="e_sb", tag="e")
                        # diagonal tile: cols < 128p fully masked, the
                        # 128-wide band [128p, 128p+128) is triangular,
                        # cols >= 128p+128 fully valid
                        c0 = 128 * p if p > 0 else 0
                        if p > 0:
                            nc.gpsimd.memset(
                                e_sb[:, :c0].bitcast(mybir.dt.uint16), 0
                            )
                        nc.scalar.activation(
                            e_sb[:, c0:], s[:, c0:], EXP, scale=SCALE, bias=EXP_OFF
                        )
                        nc.gpsimd.affine_select(
                            out=e_sb[:, c0 : c0 + 128],
                            in_=e_sb[:, c0 : c0 + 128],
                            compare_op=mybir.AluOpType.is_ge,
                            fill=0.0,
                            base=0,
                            pattern=[[1, 128]],
                            channel_multiplier=-1,
                        )
                        return e_sb

                    def o_den_mm(h, i, e_sb):
                        p = i - n_off
                        if p < 0:
                            if i % 2 == 0:
                                return  # wait for the pair to complete
                            i0 = i - 1
                            pair = e8cur[h]
                            nc.tensor.matmul(
                                o_ps[h][:],
                                lhsT=vh8[:, i0 : i0 + 2, h * 128 : (h + 1) * 128],
                                rhs=pair[:],
                                start=(i0 == 0),
                                stop=False,
                                perf_mode=DR,
                                skip_group_check=True,
                            )
                            nc.tensor.matmul(
                                den_ps[h][:],
                                lhsT=ones8[:],
                                rhs=pair[:],
                                start=(i0 == 0),
                                stop=False,
                                perf_mode=DR,
                                skip_group_check=True,
                            )
                            return
                        c0 = 128 * p if p > 0 else 0
                        nc.tensor.matmul(
                            o_ps[h][:, c0:],
                            lhsT=vh_sb[:, i, h * 128 : (h + 1) * 128],
                            rhs=e_sb[:, c0:],
                            start=(i == 0),
                            stop=(i == n_tk - 1),
                            skip_group_check=True,
                        )
                        nc.tensor.matmul(
                            den_ps[h][:, c0:],
                            lhsT=ones_sb[:],
                            rhs=e_sb[:, c0:],
                            start=(i == 0),
                            stop=(i == n_tk - 1),
                            skip_group_check=True,
                        )

                    def emit_div(h):
                        # recip = exp(-ln(den)) on ACT (DVE reciprocal is
                        # 3.3us); the x16 v scaling is divided out via
                        # w_o/16 on the host
                        lnd = rcpool.tile([128, TB], f32, name="lnd", tag="lnd")
                        nc.scalar.activation(
                            lnd[:], den_ps[h][:], mybir.ActivationFunctionType.Ln
                        )
                        recip = rcpool.tile([128, TB], f32, name="recip", tag="rcp")
                        nc.scalar.activation(recip[:], lnd[:], EXP, scale=-1.0)
                        nc.vector.tensor_mul(ocb[:, h, :], o_ps[h][:], recip[:])

                    s_pend = {0: s_mm(0, 0)}
                    for i in range(n_tk):
                        s_pend[1] = s_mm(1, i)
                        if i + 1 < n_tk:
                            s_pend[0, i + 1] = s_mm(0, i + 1)
                        e0 = exp_tile(
                            0, i, s_pend.pop(0) if i == 0 else s_pend.pop((0, i))
                        )
                        o_den_mm(0, i, e0)
                        if i == n_tk - 1:
                            # head 0 finished: divide now so its o/den psum
                            # banks free before the next block needs them
                            emit_div(0)
                        e1 = exp_tile(1, i, s_pend.pop(1))
                        o_den_mm(1, i, e1)
                        if i < n_tk - 2:
                            # last two steps stay DVE-quiet so the divide
                            # chain's ocb mul isn't queued behind copies
                            drain_pending(4)
                    emit_div(1)
                    last = b == B - 1 and j4 == NTB - 1
                    emit_proj_block(
                        b, j4, ocb, stg=1 if last else 4, alt=last
                    )
            drain_pending(len(pending))
    return nc


def prepare_inputs(x, rope_freqs, w_q, w_k, w_v, w_o):
    """Host-side sharding/layout prep. Returns per-core input maps."""
    import ml_dtypes

    bf16 = ml_dtypes.bfloat16

    x = np.asarray(x, dtype=np.float32)
    rope_freqs = np.asarray(rope_freqs, dtype=np.float32)
    w_q = np.asarray(w_q, dtype=np.float32)
    w_k = np.asarray(w_k, dtype=np.float32)
    w_v = np.asarray(w_v, dtype=np.float32)
    w_o = np.asarray(w_o, dtype=np.float32)

    xT = np.ascontiguousarray(x.transpose(0, 2, 1)).astype(bf16)  # [B, D, T]

    # permute q/k weight rows within each head: even HD idx -> rows 0..63,
    # odd -> rows 64..127 (so RoPE pairing becomes a half swap)
    perm = np.concatenate([np.arange(0, HD, 2), np.arange(1, HD, 2)])
    rows = (np.arange(D).reshape(H, HD)[:, perm]).reshape(D)
    w_qp = w_q[rows]
    w_kp = w_k[rows]

    cos = rope_freqs[..., 0].T  # [64, T]
    sin = rope_freqs[..., 1].T
    cos_sb = np.ascontiguousarray(np.concatenate([cos, cos], axis=0)).astype(
        bf16
    )  # [128, T]
    sin_sb = np.ascontiguousarray(np.concatenate([-sin, sin], axis=0)).astype(bf16)

    in_maps = []
    for cidx in range(NCORES):
        sl = slice(cidx * CD, (cidx + 1) * CD)
        in_maps.append(
            {
                "xT": xT,
                "wqT": np.ascontiguousarray(w_qp[sl].T).astype(bf16),
                "wkT": np.ascontiguousarray(w_kp[sl].T).astype(bf16),
                "wvT": np.ascontiguousarray(w_v[sl].T).astype(bf16),
                "woT": np.ascontiguousarray(w_o[:, sl].T / 16.0).astype(bf16),
                "cosd": cos_sb,
                "sind": sin_sb,
            }
        )
    return in_maps


def run(in_maps, trace=False, tmpdir=None):
    from concourse.bass_utils import run_bass_kernel_spmd

    nc = build_bass()
    res = run_bass_kernel_spmd(
        nc,
        in_maps,
        core_ids=list(range(NCORES)),
        trace=trace,
        tmpdir=tmpdir,
    )
    total = np.zeros((B, D, T), dtype=np.float32)
    for cres in res.results:
        total += np.asarray(cres["out"], dtype=np.float32)
    final = np.ascontiguousarray(total.transpose(0, 2, 1))  # [B, T, D]
    return final, res


def kernel(x, rope_freqs, w_q, w_k, w_v, w_o):
    in_maps = prepare_inputs(x, rope_freqs, w_q, w_k, w_v, w_o)
    final, _ = run(in_maps, trace=False)
    return final


# revision 47
# speedup vs baseline: 1.0177x; 1.0177x over previous
"""Causal MHA + RoPE (B=2, T=2048, D=2048, H=16, HD=128), fp32 in/out.

Tensor-parallel over heads across 8 NeuronCores (2 heads/core):
  - w_q/w_k/w_v column-sharded (rows of W), w_o row-sharded; partial
    outputs summed on the host.
  - All matmul operands are bf16 (fp32 PSUM accumulation), except the
    off-diagonal attention*V and denominator matmuls which run fp8e4
    DoubleRow (two key tiles per instruction, 2x PE throughput).
    Measured max-rel error vs the fp32 reference ~4e-3 (gate 2e-2).
  - Transposed activation layout ([feature, token]) throughout:
      qT/kT   = W_slice @ x^T              ([HD, T] per head)
      S^T     = kT.T-slice @ qT            ([tk, tq], contraction over HD)
      E       = exp(S^T*scale - 2)         (offset cancels in softmax,
                                            keeps fp8 E under e4m3 max)
      denom   = ones.T @ E                 (cross-partition sum on PE)
      O^T    += (16 v_tile).T @ E          (v scaled x16 out of fp8
                                            subnormals; divided out via
                                            the reciprocal's bias)
      partial = w_oT_slice.T @ OcatT       ([D, T] per batch, per core)
  - V is produced token-major directly (lhsT = x token-block, rhs = w_v
    slice): no PE transposes.
  - x/weights live in SBUF split into separate tiles per DMA chunk
    (Tile dependencies are whole-tile, so separate tiles let the first
    matmuls start after ~0.25 MB instead of the full buffer).
  - RoPE: q/k weight rows pre-permuted on the host (even idx -> top 64
    partitions, odd -> bottom), so the rotation is a half-swap plus
    elementwise mul/add against fp32 cos/sin tables.
"""

import numpy as np

B, T, D, H = 2, 2048, 2048, 16
HD = D // H  # 128
NCORES = 8
HPC = H // NCORES  # heads per core = 2
CD = HPC * HD  # per-core head dims = 256
SCALE = 1.0 / float(np.sqrt(HD))
TB = 512  # token block (matmul free dim)
NTB = T // TB  # 4 token blocks per batch
NKT = T // 128  # 16 key tiles per batch
KO = D // 128  # 16 contraction tiles over D
EXP_OFF = -2.0  # exp offset, cancels in softmax normalization
LN16 = float(np.log(16.0))


_PATCHED = False


def _apply_tile_patches():
    """This container's walrus build allows only ONE sync-wait command per
    TPB instruction (e.g. the S3_LW struct of a fused fp32 matmul rejects
    2 waits with "Too many sync wait commands"). Tile's scheduler freely
    puts several waits on one instruction. Two patches:

    1. After wait assignment, hoist all-but-one waits of every instruction
       onto injected same-engine NoOps placed just before it.
    2. The final TileContext drain aggregates all outstanding waits onto
       one SP Drain — split into a chain of single-wait drains.
    """
    global _PATCHED
    if _PATCHED:
        return
    _PATCHED = True

    import concourse.mybir as mybir
    import concourse.tile as tile
    from concourse.vector_clock import ScopedClock

    MAXW = 1

    _orig_lower = tile.TileContext._lower_ordered_insts

    def _lower_ordered_insts(self, ordered):
        nc = self.nc
        for insts in ordered.values():
            need = any(
                i.sync_info is not None and len(i.sync_info.on_wait) > MAXW
                for i in insts
            )
            if not need:
                continue
            out = []
            for inst in insts:
                si = inst.sync_info
                if si is not None and len(si.on_wait) > MAXW:
                    waits = list(si.on_wait)
                    extra = waits[MAXW:]
                    del si.on_wait[MAXW:]
                    for j in range(0, len(extra), MAXW):
                        nop = mybir.InstNoOp(
                            name=nc.get_next_instruction_name(), ins=[], outs=[]
                        )
                        nop.engine = inst.engine
                        nop.sync_info = mybir.SyncInfo(
                            on_wait=extra[j : j + MAXW], on_update=[]
                        )
                        nc.register_instruction(nop)
                        out.append(nop)
                out.append(inst)
            insts[:] = out
        return _orig_lower(self, ordered)

    def _drain_and_barrier(self, tick_clock, wait_clock):
        drain_inst = self.nc.sync.drain()
        wait_clock.add_sem_waits(
            drain_inst.ins, ScopedClock({None: tick_clock.global_clock})
        )
        si = drain_inst.ins.sync_info
        waits = list(si.on_wait) if si is not None else []
        if len(waits) > 1:
            del si.on_wait[1:]
            for w in waits[1:]:
                extra = self.nc.sync.drain()
                extra.ins.sync_info = mybir.SyncInfo(on_wait=[w], on_update=[])
        self.nc.all_engine_barrier()
        assert self.sems is not None
        popped = self.nc._tile_sem_poison_stack.pop()
        assert popped is self._sem_poison
        self.nc.clear_and_free_semaphores(list(self.sems.allocated().values()))
        self.nc.all_engine_barrier()

    tile.TileContext._lower_ordered_insts = _lower_ordered_insts
    tile.TileContext._drain_and_barrier = _drain_and_barrier


def build_bass():
    _apply_tile_patches()
    import concourse.bass as bass
    import concourse.mybir as mybir
    import concourse.tile as tile

    f32 = mybir.dt.float32
    bf16 = mybir.dt.bfloat16
    f8 = mybir.dt.float8e4
    EXP = mybir.ActivationFunctionType.Exp
    COPY = mybir.ActivationFunctionType.Copy
    DR = mybir.MatmulPerfMode.DoubleRow

    nc = bass.Bass("TRN2", target_bir_lowering=False, debug=False)

    xT = nc.dram_tensor("xT", [B, D, T], bf16, kind="ExternalInput").ap()
    wqT = nc.dram_tensor("wqT", [D, CD], bf16, kind="ExternalInput").ap()
    wkT = nc.dram_tensor("wkT", [D, CD], bf16, kind="ExternalInput").ap()
    wvT = nc.dram_tensor("wvT", [D, CD], bf16, kind="ExternalInput").ap()
    woT = nc.dram_tensor("woT", [CD, D], bf16, kind="ExternalInput").ap()
    cosd = nc.dram_tensor("cosd", [HD, T], bf16, kind="ExternalInput").ap()
    sind = nc.dram_tensor("sind", [HD, T], bf16, kind="ExternalInput").ap()
    out = nc.dram_tensor("out", [B, D, T], bf16, kind="ExternalOutput").ap()

    # weight/x chunk boundaries: separate tiles per chunk (Tile deps are
    # whole-tile; readers of an early chunk must not wait on later DMAs)
    WCH = ((0, 2), (2, 8), (8, KO))
    XCH0 = ((0, 4), (4, 10), (10, KO))  # first token block of a batch, finer

    with tile.TileContext(nc) as tc:
        with (
            tc.tile_pool(name="consts", bufs=1) as cpool,
            tc.tile_pool(name="acts", bufs=1) as apool,
            tc.tile_pool(name="rt", bufs=4) as rpool,
            tc.tile_pool(name="et", bufs=6) as epool,
            tc.tile_pool(name="e8t", bufs=4) as e8pool,
            tc.tile_pool(name="rc", bufs=2) as rcpool,
            tc.tile_pool(name="oc", bufs=2) as ocpool,
            tc.tile_pool(name="obp", bufs=3) as obpool,
            tc.tile_pool(name="ps", bufs=8, space="PSUM") as psp,
        ):
            # ---- persistent constants ----
            wq_t = [cpool.tile([128, k1 - k0, CD], bf16, name=f"wq{k0}") for k0, k1 in WCH]
            wk_t = [cpool.tile([128, k1 - k0, CD], bf16, name=f"wk{k0}") for k0, k1 in WCH]
            wv_sb = cpool.tile([128, KO, CD], bf16, name="wv_sb")
            wo_sb = cpool.tile([128, HPC, D], bf16, name="wo_sb")
            cos_sb = cpool.tile([128, T], bf16, name="cos_sb")
            sin_sb = cpool.tile([128, T], bf16, name="sin_sb")
            ones_sb = cpool.tile([128, 128], bf16, name="ones_sb")
            nc.vector.memset(ones_sb[:], 1.0)
            ones8 = cpool.tile([128, 2, 128], f8, name="ones8")
            nc.vector.memset(ones8[:].bitcast(mybir.dt.uint8), 0x38)  # fp8e4 1.0
            # const AP for the exp offset bias
            cap = cpool.tile([128, 1], f32, name="cexpoff")
            nc.vector.memset(cap[:], EXP_OFF)
            nc.const_aps.aps[(f32, EXP_OFF)] = cap[:]
            # warm-up matmuls: no DMA deps, ramp the PE clock to full
            # speed while the DMA engines spin up
            warm_sb = cpool.tile([128, TB], bf16, name="warm_sb")
            nc.vector.memset(warm_sb[:], 0.0)
            warm_ps = psp.tile([128, TB], f32, name="warm_ps", tag="ps")
            for _ in range(24):
                nc.tensor.matmul(
                    warm_ps[:],
                    lhsT=ones_sb[:],
                    rhs=warm_sb[:],
                    start=True,
                    stop=True,
                    skip_group_check=True,
                )

            def wqk_at(ko):
                for ci, (k0, k1) in enumerate(WCH):
                    if k0 <= ko < k1:
                        return wq_t[ci], wk_t[ci], ko - k0
                raise AssertionError

            # q/k weights stream chunk-interleaved on the ACT queue so
            # slice ko stays ahead of the matmul pace; v/o weights and
            # rope tables ride the SWDGE queue (needed later). The DMA
            # engines round-robin descriptors across every enqueued
            # transfer, so anything enqueued early steals bandwidth from
            # the first chunks — keep each queue's early contents minimal
            # and in need-order.
            def load_wqk_chunk(ci, eng):
                k0, k1 = WCH[ci]
                for w_t, src in ((wq_t[ci], wqT), (wk_t[ci], wkT)):
                    eng.dma_start(
                        w_t[:],
                        src[k0 * 128 : k1 * 128, :].rearrange(
                            "(ko p) n -> p ko n", p=128
                        ),
                    )

            def load_wv_chunk(ci):
                k0, k1 = WCH[ci]
                nc.gpsimd.dma_start(
                    wv_sb[:, k0:k1, :],
                    wvT[k0 * 128 : k1 * 128, :].rearrange(
                        "(ko p) n -> p ko n", p=128
                    ),
                )

            load_wqk_chunk(0, nc.scalar)
            load_wqk_chunk(1, nc.scalar)
            load_wv_chunk(0)
            load_wqk_chunk(2, nc.gpsimd)
            load_wv_chunk(1)
            load_wv_chunk(2)
            nc.gpsimd.dma_start(cos_sb[:], cosd)
            nc.gpsimd.dma_start(sin_sb[:], sind)
            nc.gpsimd.dma_start(
                wo_sb[:], woT.rearrange("(kk p) n -> p kk n", p=128)
            )

            # ---- per-batch activation storage (slots reused across batches) ----
            # x: one tile per DMA chunk; first token block split finer
            xb_t = {}
            for nb in range(NTB):
                chunks = XCH0 if nb == 0 else ((0, KO),)
                for h0, h1 in chunks:
                    xb_t[nb, h0, h1] = apool.tile(
                        [128, h1 - h0, TB], bf16, name=f"xb{nb}_{h0}"
                    )

            def xb_at(nb, ko):
                chunks = XCH0 if nb == 0 else ((0, KO),)
                for h0, h1 in chunks:
                    if h0 <= ko < h1:
                        return xb_t[nb, h0, h1], ko - h0
                raise AssertionError

            qT_sb = apool.tile([128, HPC, T], bf16, name="qT_sb")
            kT_sb = apool.tile([128, HPC, T], bf16, name="kT_sb")
            vh_sb = apool.tile([128, NKT, CD], bf16, name="vh_sb")
            vh8 = apool.tile([128, NKT, CD], f8, name="vh8")

            def ps_tile(nm, w=TB):
                return psp.tile([128, w], f32, name=nm, tag="ps")

            def load_x(b):
                # sync carries only what's needed soonest (nb0 low/mid,
                # then nb2/nb3); nb0-high and all of nb1 ride the ACT
                # queue interleaved with the q/k weight chunks, in
                # need-order, so no transfer is starved by round-robin
                for h0, h1 in ((0, 4), (4, 10)):
                    nc.sync.dma_start(
                        xb_t[0, h0, h1][:],
                        xT[b, h0 * 128 : h1 * 128, 0:TB].rearrange(
                            "(ko p) t -> p ko t", p=128
                        ),
                    )
                nc.scalar.dma_start(
                    xb_t[0, 10, KO][:],
                    xT[b, 10 * 128 : KO * 128, 0:TB].rearrange(
                        "(ko p) t -> p ko t", p=128
                    ),
                )
                nc.scalar.dma_start(
                    xb_t[1, 0, KO][:],
                    xT[b, :, TB : 2 * TB].rearrange("(ko p) t -> p ko t", p=128),
                )
                # xb2/xb3 are emitted later (from the v loop of nb0) with
                # a WAR-delaying touch so their descriptors don't
                # round-robin-steal DMA bandwidth from the nb0 chunks

            # pending projection work: list of thunks, each emits one
            # dout (two matmuls + copy, one store per `stg` douts)
            pending = []
            drain_on_act = [False]  # psum->sbuf copy engine for drains

            def emit_proj_block(bb, jj, ocb, stg=4, alt=False):
                tqp = slice(jj * TB, (jj + 1) * TB)
                obg = {}

                def mk(do):
                    def thunk():
                        pp = ps_tile("pp")
                        for kk in range(HPC):
                            nc.tensor.matmul(
                                pp[:],
                                lhsT=wo_sb[:, kk, do * 128 : (do + 1) * 128],
                                rhs=ocb[:, kk, :],
                                start=(kk == 0),
                                stop=(kk == HPC - 1),
                                skip_group_check=True,
                            )
                        if do % stg == 0:
                            obg["t"] = obpool.tile(
                                [128, stg, TB], bf16, name="ob", tag="ob"
                            )
                        ob = obg["t"]
                        if drain_on_act[0] or (alt and do % 2):
                            nc.scalar.copy(ob[:, do % stg, :], pp[:])
                        else:
                            nc.vector.tensor_copy(ob[:, do % stg, :], pp[:])
                        if do % stg == stg - 1:
                            seng = nc.gpsimd if bb == 0 else nc.sync
                            seng.dma_start(
                                out[
                                    bb,
                                    (do - stg + 1) * 128 : (do + 1) * 128,
                                    tqp,
                                ].rearrange("(g p) t -> p g t", p=128),
                                ob[:],
                            )

                    return thunk

                for do in range(D // 128):
                    pending.append(mk(do))

            def drain_pending(k):
                for _ in range(min(k, len(pending))):
                    pending.pop(0)()

            for b in range(B):
                load_x(b)
                # ============ QKV projections (+RoPE), V token-major ============
                for nb in range(NTB):
                    tsl = slice(nb * TB, (nb + 1) * TB)
                    psums = {}
                    for w in ("q", "k"):
                        for m in range(HPC):
                            psums[w, m] = ps_tile(f"ps_{w}{m}")
                    drain_on_act[0] = True
                    for ko in range(KO):
                        wq_c, wk_c, lko = wqk_at(ko)
                        xt, xko = xb_at(nb, ko)
                        for w, w_c in (("q", wq_c), ("k", wk_c)):
                            for m in range(HPC):
                                nc.tensor.matmul(
                                    psums[w, m][:],
                                    lhsT=w_c[:, lko, m * 128 : (m + 1) * 128],
                                    rhs=xt[:, xko, :],
                                    start=(ko == 0),
                                    stop=(ko == KO - 1),
                                )
                        if nb == 0 and ko in (5, 9, 13):
                            drain_pending(6)
                    drain_on_act[0] = False
                    # RoPE for q, k -> SBUF (all-DVE, partition-shifted
                    # reads; fp32 temps, single bf16 rounding at the add).
                    # Complete chains per head so qT/kT finish on DVE
                    # before the vh8 copies queue behind them — the
                    # attention block right below waits on these.
                    for w, dst in (("q", qT_sb), ("k", kT_sb)):
                        for m in range(HPC):
                            ps = psums[w, m]
                            tmp = rpool.tile([128, TB], f32, name="rtmp", tag="rtmp")
                            d = dst[:, m, tsl]
                            nc.vector.tensor_mul(d, ps[:], cos_sb[:, tsl])
                            nc.vector.tensor_mul(
                                tmp[0:64, :], ps[64:128, :], sin_sb[0:64, tsl]
                            )
                            nc.vector.tensor_mul(
                                tmp[64:128, :], ps[0:64, :], sin_sb[64:128, tsl]
                            )
                            nc.vector.tensor_add(d, d, tmp[:])
                    # V token-major: lhsT = x token block, rhs = w_v slice.
                    # Both copies scale v by 16 (fp8 subnormal headroom),
                    # divided back out via w_o/16 on the host.
                    for tkb in range(4):
                        pv = ps_tile("pv", w=CD)
                        for ko in range(KO):
                            xt, xko = xb_at(nb, ko)
                            nc.tensor.matmul(
                                pv[:],
                                lhsT=xt[:, xko, tkb * 128 : (tkb + 1) * 128],
                                rhs=wv_sb[:, ko, :],
                                start=(ko == 0),
                                stop=(ko == KO - 1),
                            )
                        kt = nb * 4 + tkb
                        nc.scalar.activation(
                            vh_sb[:, kt, :], pv[:], COPY, scale=16.0
                        )
                        # gpsimd (idle, SBUF-only) so the fp8 copy isn't
                        # queued behind the rope chain on DVE
                        nc.gpsimd.tensor_copy(vh8[:, kt, :], vh_sb[:, kt, :])
                        if nb == 0 and tkb in (0, 1):
                            # deferred bulk x prefetch: the gpsimd touch
                            # below executes ~25-30us in, holding the DMA
                            # back until the first block's chunks landed
                            xnb = 2 + tkb
                            nc.gpsimd.memset(
                                xb_t[xnb, 0, KO][:, 0, 0:1].bitcast(
                                    mybir.dt.uint16
                                ),
                                0,
                            )
                            nc.sync.dma_start(
                                xb_t[xnb, 0, KO][:],
                                xT[
                                    b, :, xnb * TB : (xnb + 1) * TB
                                ].rearrange("(ko p) t -> p ko t", p=128),
                            )

                    # ======== attention block j4 = nb (keys 0..nb ready) ========
                    # interleaved with QKV so the PE stays dense while x
                    # and weights stream in, and so block-end divide
                    # chains overlap the next block's projections
                    j4 = nb
                    tq = slice(j4 * TB, (j4 + 1) * TB)
                    n_tk = 4 * (j4 + 1)
                    n_off = 4 * j4  # off-diagonal tiles (fp8 DoubleRow pairs)
                    ocb = ocpool.tile([128, HPC, TB], bf16, name="ocb", tag="ocb")
                    o_ps = [ps_tile(f"o_ps{h}") for h in range(HPC)]
                    den_ps = [ps_tile(f"den_ps{h}") for h in range(HPC)]
                    e8cur = {}

                    def s_mm(h, i):
                        s = ps_tile("s_ps")
                        p = i - n_off
                        c0 = 128 * p if p > 0 else 0
                        nc.tensor.matmul(
                            s[:, c0:],
                            lhsT=kT_sb[:, h, i * 128 : (i + 1) * 128],
                            rhs=qT_sb[:, h, j4 * TB + c0 : (j4 + 1) * TB],
                            start=True,
                            stop=True,
                            skip_group_check=True,
                        )
                        return s

                    def exp_tile(h, i, s):
                        p = i - n_off
                        if p < 0:
                            # off-diagonal: fp8 pair slot for DoubleRow
                            if i % 2 == 0:
                                e8cur[h] = e8pool.tile(
                                    [128, 2, TB], f8, name="e8", tag="e8"
                                )
                            e_sb = e8cur[h][:, i % 2, :]
                            nc.scalar.activation(
                                e_sb, s[:], EXP, scale=SCALE, bias=EXP_OFF
                            )
                            return e_sb
                        e_sb = epool.tile([128, TB], bf16, name="e_sb", tag="e")
                        # diagonal tile: cols < 128p fully masked, the
                        # 128-wide band [128p, 128p+128) is triangular,
                        # cols >= 128p+128 fully valid
                        c0 = 128 * p if p > 0 else 0
                        if p > 0:
                            nc.gpsimd.memset(
                                e_sb[:, :c0].bitcast(mybir.dt.uint16), 0
                            )
                        nc.scalar.activation(
                            e_sb[:, c0:], s[:, c0:], EXP, scale=SCALE, bias=EXP_OFF
                        )
                        nc.gpsimd.affine_select(
                            out=e_sb[:, c0 : c0 + 128],
                            in_=e_sb[:, c0 : c0 + 128],
                            compare_op=mybir.AluOpType.is_ge,
                            fill=0.0,
                            base=0,
                            pattern=[[1, 128]],
                            channel_multiplier=-1,
                        )
                        return e_sb

                    def o_den_mm(h, i, e_sb):
                        p = i - n_off
                        if p < 0:
                            if i % 2 == 0:
                                return  # wait for the pair to complete
                            i0 = i - 1
                            pair = e8cur[h]
                            nc.tensor.matmul(
                                o_ps[h][:],
                                lhsT=vh8[:, i0 : i0 + 2, h * 128 : (h + 1) * 128],
                                rhs=pair[:],
                                start=(i0 == 0),
                                stop=False,
                                perf_mode=DR,
                                skip_group_check=True,
                            )
                            nc.tensor.matmul(
                                den_ps[h][:],
                                lhsT=ones8[:],
                                rhs=pair[:],
                                start=(i0 == 0),
                                stop=False,
                                perf_mode=DR,
                                skip_group_check=True,
                            )
                            return
                        c0 = 128 * p if p > 0 else 0
                        nc.tensor.matmul(
                            o_ps[h][:, c0:],
                            lhsT=vh_sb[:, i, h * 128 : (h + 1) * 128],
                            rhs=e_sb[:, c0:],
                            start=(i == 0),
                            stop=(i == n_tk - 1),
                            skip_group_check=True,
                        )
                        nc.tensor.matmul(
                            den_ps[h][:, c0:],
                            lhsT=ones_sb[:],
                            rhs=e_sb[:, c0:],
                            start=(i == 0),
                            stop=(i == n_tk - 1),
                            skip_group_check=True,
                        )

                    def emit_div(h):
                        # recip = exp(-ln(den)) on ACT (DVE reciprocal is
                        # 3.3us); the x16 v scaling is divided out via
                        # w_o/16 on the host
                        lnd = rcpool.tile([128, TB], f32, name="lnd", tag="lnd")
                        nc.scalar.activation(
                            lnd[:], den_ps[h][:], mybir.ActivationFunctionType.Ln
                        )
                        recip = rcpool.tile([128, TB], f32, name="recip", tag="rcp")
                        nc.scalar.activation(recip[:], lnd[:], EXP, scale=-1.0)
                        nc.vector.tensor_mul(ocb[:, h, :], o_ps[h][:], recip[:])

                    s_pend = {0: s_mm(0, 0)}
                    for i in range(n_tk):
                        s_pend[1] = s_mm(1, i)
                        if i + 1 < n_tk:
                            s_pend[0, i + 1] = s_mm(0, i + 1)
                        e0 = exp_tile(
                            0, i, s_pend.pop(0) if i == 0 else s_pend.pop((0, i))
                        )
                        o_den_mm(0, i, e0)
                        if i == n_tk - 1:
                            # head 0 finished: divide now so its o/den psum
                            # banks free before the next block needs them
                            emit_div(0)
                        e1 = exp_tile(1, i, s_pend.pop(1))
                        o_den_mm(1, i, e1)
                        if i < n_tk - 2:
                            # last two steps stay DVE-quiet so the divide
                            # chain's ocb mul isn't queued behind copies
                            drain_pending(4)
                    emit_div(1)
                    last = b == B - 1 and j4 == NTB - 1
                    emit_proj_block(b, j4, ocb, stg=2 if last else 4)
            drain_pending(len(pending))
    return nc


def prepare_inputs(x, rope_freqs, w_q, w_k, w_v, w_o):
    """Host-side sharding/layout prep. Returns per-core input maps."""
    import ml_dtypes

    bf16 = ml_dtypes.bfloat16

    x = np.asarray(x, dtype=np.float32)
    rope_freqs = np.asarray(rope_freqs, dtype=np.float32)
    w_q = np.asarray(w_q, dtype=np.float32)
    w_k = np.asarray(w_k, dtype=np.float32)
    w_v = np.asarray(w_v, dtype=np.float32)
    w_o = np.asarray(w_o, dtype=np.float32)

    xT = np.ascontiguousarray(x.transpose(0, 2, 1)).astype(bf16)  # [B, D, T]

    # permute q/k weight rows within each head: even HD idx -> rows 0..63,
    # odd -> rows 64..127 (so RoPE pairing becomes a half swap)
    perm = np.concatenate([np.arange(0, HD, 2), np.arange(1, HD, 2)])
    rows = (np.arange(D).reshape(H, HD)[:, perm]).reshape(D)
    w_qp = w_q[rows]
    w_kp = w_k[rows]

    cos = rope_freqs[..., 0].T  # [64, T]
    sin = rope_freqs[..., 1].T
    cos_sb = np.ascontiguousarray(np.concatenate([cos, cos], axis=0)).astype(
        bf16
    )  # [128, T]
    sin_sb = np.ascontiguousarray(np.concatenate([-sin, sin], axis=0)).astype(bf16)

    in_maps = []
    for cidx in range(NCORES):
        sl = slice(cidx * CD, (cidx + 1) * CD)
        in_maps.append(
            {
                "xT": xT,
                "wqT": np.ascontiguousarray(w_qp[sl].T).astype(bf16),
                "wkT": np.ascontiguousarray(w_kp[sl].T).astype(bf16),
                "wvT": np.ascontiguousarray(w_v[sl].T).astype(bf16),
                "woT": np.ascontiguousarray(w_o[:, sl].T / 16.0).astype(bf16),
                "cosd": cos_sb,
                "sind": sin_sb,
            }
        )
    return in_maps


def run(in_maps, trace=False, tmpdir=None):
    from concourse.bass_utils import run_bass_kernel_spmd

    nc = build_bass()
    res = run_bass_kernel_spmd(
        nc,
        in_maps,
        core_ids=list(range(NCORES)),
        trace=trace,
        tmpdir=tmpdir,
    )
    total = np.zeros((B, D, T), dtype=np.float32)
    for cres in res.results:
        total += np.asarray(cres["out"], dtype=np.float32)
    final = np.ascontiguousarray(total.transpose(0, 2, 1))  # [B, T, D]
    return final, res


def kernel(x, rope_freqs, w_q, w_k, w_v, w_o):
    in_maps = prepare_inputs(x, rope_freqs, w_q, w_k, w_v, w_o)
    final, _ = run(in_maps, trace=False)
    return final
